# revision 3
# baseline (speedup 1.0000x reference)
"""Bass/Trainium2 kernel for nn_Attention_27960237097134.

Additive-attention forward:
    h_proj = hidden @ W_h.T + b_h                      [B, A]
    a_proj = features @ W_a.T + b_a                    [B, T, A]
    energy = (tanh(h_proj + a_proj) @ v_w[0] + v_b)/sqrt(A)   [B, T]
    alpha  = softmax(energy, axis=1)
    context = einsum('bt,btf->bf', alpha, features)
Returns (context, alpha).

Strategy: data-parallel over batch (4 batches per core x 8 cores). Per
batch, features stream through SBUF once in four t-chunks of 512 rows.
Each chunk is PE-transposed into [f, t] tiles feeding a float32r
(full-rate, ~tf32-precision) matmul against resident W_a^T; tanh runs
on ScalarE with the per-partition h_proj bias folded in; energy is a
v-stationary matmul; softmax is streamed (exp unnormalized, |energy| is
provably < ~1.4 so no max subtraction); context accumulates per-chunk
matmul partials, normalized at the end.
"""
import math

import numpy as np

import concourse.bass as bass
import concourse.mybir as mybir
import concourse.tile as tile
from concourse import bacc
from concourse import bass_utils
from concourse.masks import make_identity

B, T, F, H, A = 32, 2048, 2048, 1024, 512
NCORES = 8
BL = B // NCORES          # batches per core
TC = 512                  # t-chunk
NTC = T // TC             # chunks per batch
NTS = TC // 128           # t-subtiles per chunk
NFT = F // 128            # f k-tiles
NFC = F // 512            # f chunks for context
NAT = A // 128            # a tiles
NHT = H // 128            # h k-tiles

f32 = mybir.dt.float32
f32r = mybir.dt.float32r
bf16 = mybir.dt.bfloat16
AF = mybir.ActivationFunctionType

_CACHED_NC = None


def _build():
    nc = bacc.Bacc("TRN2", target_bir_lowering=False, debug=False,
                   num_devices=NCORES)
    feat_d = nc.dram_tensor("features", [BL, T, F], f32r, kind="ExternalInput")
    wat_d = nc.dram_tensor("wat", [F, A], f32r, kind="ExternalInput")
    wht_d = nc.dram_tensor("wht", [H, A], f32, kind="ExternalInput")
    hid_d = nc.dram_tensor("hiddenT", [H, BL], f32, kind="ExternalInput")
    bias_d = nc.dram_tensor("bias_ha", [A], f32, kind="ExternalInput")
    v_d = nc.dram_tensor("v_scaled", [A], f32, kind="ExternalInput")
    vb_d = nc.dram_tensor("vb_scaled", [1, 1], f32, kind="ExternalInput")
    ctx_d = nc.dram_tensor("context", [BL, F], f32, kind="ExternalOutput")
    alpha_d = nc.dram_tensor("alpha", [BL, T], f32, kind="ExternalOutput")

    with tile.TileContext(nc) as tc:
        with tc.tile_pool(name="sbuf", bufs=1) as pool, \
             tc.tile_pool(name="psum", bufs=1, space="PSUM") as pp:
            # --- constants / weights ---
            ident = pool.tile([128, 128], f32)
            make_identity(nc, ident[:])
            identr = pool.tile([128, 128], f32r)
            nc.vector.tensor_copy(out=identr[:], in_=ident[:])

            wat_sb = pool.tile([128, NFT * A], f32r)
            nc.sync.dma_start(
                out=wat_sb[:].rearrange("p (ft a) -> p ft a", ft=NFT),
                in_=wat_d.ap().rearrange("(ft p) a -> p ft a", p=128))
            wht_sb = pool.tile([128, NHT * A], f32)
            nc.sync.dma_start(
                out=wht_sb[:].rearrange("p (ht a) -> p ht a", ht=NHT),
                in_=wht_d.ap().rearrange("(ht p) a -> p ht a", p=128))
            hid_sb = pool.tile([128, NHT * BL], f32)
            nc.sync.dma_start(
                out=hid_sb[:].rearrange("p (ht b) -> p ht b", ht=NHT),
                in_=hid_d.ap().rearrange("(ht p) b -> p ht b", p=128))
            bias_sb = pool.tile([128, NAT], f32)
            nc.sync.dma_start(
                out=bias_sb[:],
                in_=bias_d.ap().rearrange("(at p) -> p at", p=128))
            v_sb = pool.tile([128, NAT], f32)
            nc.sync.dma_start(
                out=v_sb[:],
                in_=v_d.ap().rearrange("(at p) -> p at", p=128))
            v_bf = pool.tile([128, NAT], bf16)
            nc.vector.tensor_copy(out=v_bf[:], in_=v_sb[:])
            vb_sb = pool.tile([1, 1], f32)
            nc.sync.dma_start(out=vb_sb[:], in_=vb_d.ap())

            # --- h_proj for all local batches: hb[a, at*BL+b] ---
            hp_ps = pp.tile([128, NAT * BL], f32, tag="small", bufs=2)
            for at in range(NAT):
                for ht in range(NHT):
                    nc.tensor.matmul(
                        hp_ps[:, at * BL:(at + 1) * BL],
                        wht_sb[:, ht * A + at * 128:ht * A + (at + 1) * 128],
                        hid_sb[:, ht * BL:(ht + 1) * BL],
                        start=(ht == 0), stop=(ht == NHT - 1))
            hb_sb = pool.tile([128, NAT * BL], f32)
            for at in range(NAT):
                nc.scalar.add(hb_sb[:, at * BL:(at + 1) * BL],
                              hp_ps[:, at * BL:(at + 1) * BL],
                              bias_sb[:, at:at + 1])

            # --- main loop ---
            for b in range(BL):
                alpha_row = pool.tile([1, T], f32, tag="alpha_row", bufs=2,
                                      name=f"alpha_row{b}")
                esums = pool.tile([1, NTC], f32, tag="esums", bufs=2,
                                  name=f"esums{b}")
                ctx_acc = pool.tile([1, F], f32, tag="ctx_acc", bufs=2,
                                    name=f"ctx_acc{b}")
                for c in range(NTC):
                    # load chunk: fn[p, ts*F + f] = feat[b, c*TC+ts*128+p, f]
                    fn = pool.tile([128, NTS * F], f32r, tag="fn", bufs=2,
                                   name=f"fn{b}_{c}")
                    nc.sync.dma_start(
                        out=fn[:].rearrange("p (ts f) -> p ts f", ts=NTS),
                        in_=feat_d.ap()[b, c * TC:(c + 1) * TC, :]
                        .rearrange("(ts p) f -> p ts f", p=128))

                    # transpose into featT[p, ft*TC + t]
                    featT = pool.tile([128, NFT * TC], f32r, tag="featT",
                                      bufs=1, name=f"featT{b}_{c}")
                    for ft in range(NFT):
                        scr = pp.tile([128, TC], f32r, tag="scr", bufs=2,
                                      name=f"scr{b}_{c}_{ft}")
                        for ts in range(NTS):
                            nc.tensor.transpose(
                                scr[:, ts * 128:(ts + 1) * 128],
                                fn[:, ts * F + ft * 128:ts * F + (ft + 1) * 128],
                                identr[:])
                        if ft % 2 == 0:
                            nc.scalar.copy(
                                out=featT[:, ft * TC:(ft + 1) * TC], in_=scr[:])
                        else:
                            nc.vector.tensor_copy(
                                out=featT[:, ft * TC:(ft + 1) * TC], in_=scr[:])

                    # a_projT[a, t] += W_aT k-tiles
                    aps = pp.tile([128, NAT * TC], f32, tag="aproj", bufs=1,
                                  name=f"aps{b}_{c}")
                    for at in range(NAT):
                        for ft in range(NFT):
                            nc.tensor.matmul(
                                aps[:, at * TC:(at + 1) * TC],
                                wat_sb[:, ft * A + at * 128:ft * A + (at + 1) * 128],
                                featT[:, ft * TC:(ft + 1) * TC],
                                start=(ft == 0), stop=(ft == NFT - 1))

                    # tanh(a_proj + h_proj) -> bf16
                    th = pool.tile([128, NAT * TC], bf16, tag="th", bufs=2,
                                   name=f"th{b}_{c}")
                    for at in range(NAT):
                        nc.scalar.activation(
                            th[:, at * TC:(at + 1) * TC],
                            aps[:, at * TC:(at + 1) * TC],
                            AF.Tanh, bias=hb_sb[:, at * BL + b:at * BL + b + 1])

                    # energy[1, t]
                    eps = pp.tile([1, TC], f32, tag="small", bufs=2,
                                  name=f"eps{b}_{c}")
                    for at in range(NAT):
                        nc.tensor.matmul(
                            eps[:], v_bf[:, at:at + 1],
                            th[:, at * TC:(at + 1) * TC],
                            start=(at == 0), stop=(at == NAT - 1))

                    # exp + running sum
                    nc.scalar.activation(
                        alpha_row[0:1, c * TC:(c + 1) * TC], eps[:],
                        AF.Exp, bias=vb_sb[:],
                        accum_out=esums[0:1, c:c + 1])

                    # alpha column tiles [t,1] via PE transpose
                    atp = pp.tile([128, NTS], f32, tag="small", bufs=2,
                                  name=f"atp{b}_{c}")
                    for ts in range(NTS):
                        nc.tensor.transpose(
                            atp[:, ts:ts + 1],
                            alpha_row[0:1, c * TC + ts * 128:c * TC + (ts + 1) * 128],
                            ident[0:1, 0:1])
                    alphaT = pool.tile([128, NTS], f32r, tag="alphaT", bufs=2,
                                       name=f"alphaT{b}_{c}")
                    nc.vector.tensor_copy(out=alphaT[:], in_=atp[:])

                    # context partials
                    for fc in range(NFC):
                        cps = pp.tile([1, 512], f32, tag="small", bufs=2,
                                      name=f"cps{b}_{c}_{fc}")
                        for ts in range(NTS):
                            nc.tensor.matmul(
                                cps[:], alphaT[:, ts:ts + 1],
                                fn[:, ts * F + fc * 512:ts * F + (fc + 1) * 512],
                                start=(ts == 0), stop=(ts == NTS - 1))
                        if c == 0:
                            nc.vector.tensor_copy(
                                out=ctx_acc[0:1, fc * 512:(fc + 1) * 512],
                                in_=cps[:])
                        else:
                            nc.vector.tensor_add(
                                out=ctx_acc[0:1, fc * 512:(fc + 1) * 512],
                                in0=ctx_acc[0:1, fc * 512:(fc + 1) * 512],
                                in1=cps[:])

                # normalize + store
                tot = pool.tile([1, 1], f32, tag="tot", bufs=2, name=f"tot{b}")
                nc.vector.reduce_sum(tot[:], esums[:], axis=mybir.AxisListType.X)
                s = pool.tile([1, 1], f32, tag="s", bufs=2, name=f"s{b}")
                nc.vector.reciprocal(s[:], tot[:])
                nc.scalar.activation(ctx_acc[:], ctx_acc[:], AF.Copy, scale=s[:])
                nc.scalar.activation(alpha_row[:], alpha_row[:], AF.Copy,
                                     scale=s[:])
                nc.sync.dma_start(out=ctx_d.ap()[b:b + 1, :], in_=ctx_acc[:])
                nc.sync.dma_start(out=alpha_d.ap()[b:b + 1, :], in_=alpha_row[:])

    nc.compile()
    return nc


def _get_nc():
    global _CACHED_NC
    if _CACHED_NC is None:
        _CACHED_NC = _build()
    return _CACHED_NC


def make_in_maps(features, hidden, W_h, b_h, W_a, b_a, v_w, v_b):
    features = np.ascontiguousarray(np.asarray(features, dtype=np.float32))
    hidden = np.asarray(hidden, dtype=np.float32)
    wat = np.ascontiguousarray(np.asarray(W_a, dtype=np.float32).T)
    wht = np.ascontiguousarray(np.asarray(W_h, dtype=np.float32).T)
    bias_ha = (np.asarray(b_h, dtype=np.float32)
               + np.asarray(b_a, dtype=np.float32))
    scale = np.float32(1.0 / math.sqrt(A))
    v_scaled = np.asarray(v_w, dtype=np.float32)[0] * scale
    vb_scaled = (np.asarray(v_b, dtype=np.float32) * scale).reshape(1, 1)
    in_maps = []
    for c in range(NCORES):
        sl = slice(c * BL, (c + 1) * BL)
        in_maps.append({
            "features": features[sl],
            "wat": wat,
            "wht": wht,
            "hiddenT": np.ascontiguousarray(hidden[sl].T),
            "bias_ha": bias_ha,
            "v_scaled": v_scaled,
            "vb_scaled": vb_scaled,
        })
    return in_maps


def kernel(features, hidden, W_h, b_h, W_a, b_a, v_w, v_b):
    nc = _get_nc()
    in_maps = make_in_maps(features, hidden, W_h, b_h, W_a, b_a, v_w, v_b)
    res = bass_utils.run_bass_kernel_spmd(nc, in_maps,
                                          core_ids=list(range(NCORES)))
    context = np.concatenate([res.results[c]["context"]
                              for c in range(NCORES)], axis=0)
    alpha = np.concatenate([res.results[c]["alpha"]
                            for c in range(NCORES)], axis=0)
    return context.astype(np.float32), alpha.astype(np.float32)
